# revision 60
# baseline (speedup 1.0000x reference)
"""Trainium2 Bass kernel for nn_MessageLayer (GNN message passing), 8 NeuronCores.

Reference computation:
    edge_mat = (edge_features @ W + b).reshape(E, 64, 16)
    messages = einsum('emh,eh->em', edge_mat, hidden[edge_sources])
    out      = segment_sum(messages, edge_targets, num_segments=10000)

Algebraic restructure (cuts FLOPs 32x): since aggregation is linear,
    out[n, m] = sum_{f,h} W[f, m*16+h] * C[n, f, h],
    C[n, f, h] = sum_{e: tgt(e)=n} ef[e, f] * hidden[src(e), h]
Then out = C @ Wr as 32 accumulating matmuls against a block-diagonal W.

C-stage structure: the PE issue rate (~35 ns per matmul call, nearly
independent of operand size) dominates, and the block-diag nh operand
ships NPOS x 4 KB of mostly zeros, so the segment count is minimized:
natural segments (max degree 56, no splitting) packed THREE to a 128-row
tile at TIGHT cumulative row offsets (the block-diag separation is by
columns, so groups need no row alignment; rank-spaced triplets keep every
tile's row sum <= 128).  Per tile ONE matmul:
    lhsT = ef   [rows, 32]   (compact edge features, all three groups)
    rhs  = nhbd [rows, 48]   (block-diagonal: group g's source-hidden in
                              cols 16g..16g+16, zeros elsewhere)
    out  = [32f, 48=(g,h)] in PSUM at partition group 32q, q from tile idx
so C for three segments lands in one PSUM write.  All matmuls contract from
row 0 (same PE row-group) so they serialize — no concurrent PSUM-bank
drains (the known wedge).  The block-diagonal rhs is packed on the host
(zeros ship from HBM; still cheaper than 3x the matmul calls).

All matmul operands are bf16 (PSUM stays fp32): 1 cycle/row vs fp32's 4,
half the DMA.  Quantization error ~0.3% vs the 2e-2 gate.

Sharding: node-ownership, no collective.  Nodes are dealt snake-wise in
descending-degree order so per-core sorted segment profiles match and the
SPMD cross-core max-padding (K_j) is minimal.

c_all is h-major so the W-stage moving operand c_all[:, h, :] is contiguous
(a strided moving was measured 3x slower).  Input chunks alternate between
the two HW DGE queues (SP + Activation); the kernel is DMA-bound (~9.6 MB at
~420 GB/s shared HBM), so the W stage is split into passes that run while
the PE would otherwise stall waiting for late chunks, and wbd ships early
in the stream.  Exec time: 156 us (fp32 baseline) -> ~47-50 us measured.
"""
import numpy as np
import ml_dtypes
from contextlib import ExitStack

BF16 = ml_dtypes.bfloat16

N_NODES = 10000
N_EDGES = 320000
HID = 16
MSG = 64
EFD = 32
NCORES = 8
SPT = 3                          # segments per tile (tight row packing)
BPT = 40                         # tiles per PSUM bank (4q x 10 slots)
CPB = SPT * (BPT // 4)           # c_all cols per bank (10 slots x 3 groups)
RW = EFD + SPT * HID             # 80 packed cols per row: ef | nhbd(3x16)
# progressive input chunks: small first chunk so the PE starts early, taper
# at the end so the last casts (which gate the W stage) come early
CHUNK_FRACS = (0.0, 0.015, 0.05, 0.10, 0.16, 0.23, 0.31, 0.40, 0.49, 0.58,
               0.67, 0.76, 0.85, 0.93, 1.0)
NCHUNK = len(CHUNK_FRACS) - 1
WBD_AFTER = 1                    # ship wbd after this chunk (needed by W-A)
W_SPLITS = (3, 6, 9)             # early W passes after these bank counts

_CACHE = {}


def _build_layout(edge_targets):
    """Per-core segment lists (<=64 edges each, K-sorted) + SPMD-uniform K."""
    deg = np.bincount(edge_targets, minlength=N_NODES)
    order = np.argsort(-deg, kind="stable")      # nodes by degree desc
    node_core = np.empty(N_NODES, dtype=np.int64)
    snake = list(range(NCORES)) + list(range(NCORES - 1, -1, -1))
    for i, n in enumerate(order):
        node_core[n] = snake[i % (2 * NCORES)]

    order_e = np.argsort(edge_targets, kind="stable")
    tgt_sorted = edge_targets[order_e]
    uniq, starts = np.unique(tgt_sorted, return_index=True)
    bounds = list(starts) + [len(tgt_sorted)]

    segs_per_core = [[] for _ in range(NCORES)]
    for i, n in enumerate(uniq):
        s, e = bounds[i], bounds[i + 1]
        c = node_core[n]
        while e - s > 64:           # split to <=64; host re-adds partials
            segs_per_core[c].append((int(n), order_e[s:s + 64]))
            s += 64
        segs_per_core[c].append((int(n), order_e[s:e]))
    for c in range(NCORES):
        segs_per_core[c].sort(key=lambda t: -len(t[1]))

    NPOS = max(len(s) for s in segs_per_core)
    NPOS = ((NPOS + SPT - 1) // SPT) * SPT
    K = np.ones(NPOS, dtype=np.int64)
    for segs in segs_per_core:
        for j, (_, e) in enumerate(segs):
            K[j] = max(K[j], len(e))
    T = NPOS // SPT
    # rank-spaced quintets: tile t holds positions {g*T + t}; with K sorted
    # desc the per-tile row sums stay well under 128
    assert _offsets(K, T)[:, SPT].max() <= 128
    NB = (T + BPT - 1) // BPT
    return segs_per_core, NPOS, K, T, NB


def _offsets(K, T):
    # OFF[t, g] = start row of group g in tile t (tight cumulative packing)
    ks = K.reshape(SPT, T)                       # [g, t]
    off = np.zeros((T, SPT + 1), dtype=np.int64)
    off[:, 1:] = np.cumsum(ks.T, axis=1)
    return off


def _pack_core(segs, NPOS, K, T, wbd, edge_features, edge_sources, hidden):
    # position j -> tile t = j %% T, group g = j // T, rows tightly packed at
    # cumulative offsets OFF[t, g] (block-diag separation is by columns, so
    # groups need no row alignment)
    off = _offsets(K, T)
    d = np.zeros((T * 128, RW), dtype=np.float32)
    for j in range(min(len(segs), NPOS)):
        _, eids = segs[j]
        g, t = j // T, j % T
        base = t * 128 + off[t, g]
        d[base:base + len(eids), :EFD] = edge_features[eids]
        d[base:base + len(eids), EFD + HID * g:EFD + HID * (g + 1)] = \
            hidden[edge_sources[eids]]
    # DRAM [128 partitions, T*112 + wbd]: tile t at free offset 112t
    d = d.reshape(T, 128, RW).swapaxes(0, 1).reshape(128, T * RW)
    return np.ascontiguousarray(np.concatenate([d.astype(BF16), wbd], axis=1))


def _build_wrep(W):
    # Wbd[p=2h+half] [(q,f)=128, (q',mh)=128] = delta_qq' W[f, (mh+32*half)*16+h]
    wbd = np.zeros((32, 128, 128), dtype=np.float32)
    Wr = W.reshape(EFD, MSG, HID)                      # [f, m, h]
    for h in range(HID):
        for half in range(2):
            p = 2 * h + half
            blk = Wr[:, 32 * half:32 * half + 32, h]   # [f=32, mh=32]
            for q in range(4):
                wbd[p, 32 * q:32 * q + 32, 32 * q:32 * q + 32] = blk
    # DRAM layout [128, 32*128]: phase p at free offset 128p
    return np.ascontiguousarray(
        wbd.transpose(1, 0, 2).reshape(128, 32 * 128)).astype(BF16)


def _chunk_bounds(T):
    b = [round(f * T) for f in CHUNK_FRACS]
    b[-1] = T
    return b


def _build_program(NPOS, K, T, NB):
    import concourse.tile as tile
    from concourse import bacc, mybir

    f32 = mybir.dt.float32
    bf16 = mybir.dt.bfloat16
    bounds = _chunk_bounds(T)

    nc = bacc.Bacc("TRN2", target_bir_lowering=False, debug=False,
                   num_devices=NCORES)
    data_dram = nc.dram_tensor("data", [128, T * RW + 32 * 128], bf16,
                               kind="ExternalInput").ap()
    out_dram = nc.dram_tensor("out", [128, 2 * NB * CPB], f32,
                              kind="ExternalOutput").ap()
    off = _offsets(K, T)

    with tile.TileContext(nc) as tc, ExitStack() as ctx:
        big = ctx.enter_context(tc.tile_pool(name="big", bufs=1))
        cpool = ctx.enter_context(tc.tile_pool(name="cps", bufs=3,
                                               space="PSUM"))
        opool = ctx.enter_context(tc.tile_pool(name="ops", bufs=2, space="PSUM"))

        # wbd rides the otherwise-idle GpSimd software-DGE path so the
        # two HW queues carry nothing but input chunks; it only has to
        # land before the first W pass (~16 us in)
        wbd_sb = big.tile([128, 32 * 128], bf16, tag="wbd", name="wbd")
        nc.gpsimd.dma_start(wbd_sb[:], data_dram[:, T * RW:])

        ch_sb = []
        for k in range(NCHUNK):
            lo, hi = bounds[k] * RW, bounds[k + 1] * RW
            t = big.tile([128, hi - lo], bf16, tag=f"ch{k}", name=f"ch{k}")
            # full-width transfers, alternating HW DGE queues (partial-
            # partition DMAs measured ~60 GB/s -- a descriptor slow-path --
            # so the row-padding ships despite being dead weight)
            eng = nc.sync if k % 2 == 0 else nc.scalar
            eng.dma_start(t[:], data_dram[:, lo:hi])
            ch_sb.append(t)

        # h-major: c_all[p, h, 32*b + w2], w2 = 4*j8 + g.  Columns of the
        # partial last bank with no backing tile carry PSUM garbage; the
        # W-stage is column-independent and _assemble never reads them.
        c_all = big.tile([128, HID, NB * CPB], bf16, tag="call")

        out_sb = big.tile([128, 2 * NB * CPB], f32, tag="outsb")

        def w_stage(lo_b, hi_b):
            # out = C @ Wr restricted to banks [lo_b, hi_b).
            # po is a full PSUM bank: two sub-bank po tiles can share a
            # bank, and a start=True reset clobbers the bank-mate.
            lo, hi = CPB * lo_b, CPB * hi_b
            for half in range(2):
                po = opool.tile([128, 512], f32, tag="po",
                                name=f"po{lo_b}_{half}")[:, :hi - lo]
                for h in range(HID):
                    p = 2 * h + half
                    nc.tensor.matmul(
                        po[:], wbd_sb[:, 128 * p:128 * p + 128],
                        c_all[:, h, lo:hi],
                        start=(h == 0), stop=(h == HID - 1))
                sl = slice(NB * CPB * half + lo, NB * CPB * half + hi)
                if half == 0:
                    nc.vector.tensor_copy(out_sb[:, sl], po[:])
                else:
                    nc.scalar.copy(out_sb[:, sl], po[:])
                nc.gpsimd.dma_start(out_dram[:, sl], out_sb[:, sl])

        chunk_of = np.searchsorted(np.array(bounds[1:]), np.arange(T),
                                   side="right")
        cps = None
        for t in range(T):
            if cps is None:
                cps = cpool.tile([128, BPT // 4, SPT, HID], f32, tag="cps",
                                 name=f"cps_b{t // BPT}")
            ch = int(chunk_of[t])
            base = (t - bounds[ch]) * RW
            idx = t % BPT
            q, j6 = idx % 4, idx // 4
            kk = int(off[t, SPT])
            lhsT = ch_sb[ch][0:kk, base:base + EFD]
            rhs = ch_sb[ch][0:kk, base + EFD:base + RW]
            out = cps[32 * q:32 * q + 32, j6, :, :]
            nc.tensor.matmul(out, lhsT, rhs, start=True, stop=True,
                             tile_position=(0, 32 * q))
            if t % BPT == BPT - 1 or t == T - 1:
                b = t // BPT
                src = cps[:, :, :, :].transpose([0, 3, 1, 2])
                if b % 2 == 0:
                    nc.vector.tensor_copy(c_all[:, :, CPB * b:CPB * (b + 1)],
                                          src)
                else:
                    nc.scalar.copy(c_all[:, :, CPB * b:CPB * (b + 1)], src)
                cps = None
                # early W passes while the PE would otherwise stall
                # waiting on late input chunks
                if b + 1 in W_SPLITS:
                    i = W_SPLITS.index(b + 1)
                    w_stage(W_SPLITS[i - 1] if i else 0, b + 1)
        w_stage(W_SPLITS[-1], NB)
    nc.compile()
    return nc


def _assemble(outs, segs_per_core, NPOS, NB):
    WND = NB * CPB
    out = np.zeros((N_NODES, MSG), dtype=np.float32)
    T = NPOS // SPT
    j = np.arange(NPOS)
    g = j // T
    t = j % T
    b = t // BPT
    idx = t % BPT
    q = idx % 4
    j6 = idx // 4
    col = CPB * b + SPT * j6 + g
    for c in range(NCORES):
        out_sb = outs[c]
        pos_rows = np.empty((NPOS, MSG), dtype=np.float32)
        for half in range(2):
            pos_rows[:, 32 * half:32 * half + 32] = \
                out_sb[32 * q[:, None] + np.arange(32)[None, :],
                       (WND * half + col)[:, None]]
        segs = segs_per_core[c]
        for jj in range(min(len(segs), NPOS)):
            n, _ = segs[jj]
            out[n] += pos_rows[jj]
    return out


def kernel(node_features, edge_features, edge_sources, edge_targets,
           hidden, initial, W, b):
    from concourse.bass_utils import run_bass_kernel_spmd

    edge_targets = np.asarray(edge_targets)
    edge_sources = np.asarray(edge_sources)
    edge_features = np.asarray(edge_features, dtype=np.float32)
    hidden = np.asarray(hidden, dtype=np.float32)
    W = np.asarray(W, dtype=np.float32)
    b = np.asarray(b, dtype=np.float32)

    key = edge_targets.tobytes()
    if key in _CACHE:
        layout, nc = _CACHE[key]
    else:
        layout = _build_layout(edge_targets)
        segs_per_core, NPOS, K, T, NB = layout
        assert K.max() <= 64
        nc = _build_program(NPOS, K, T, NB)
        _CACHE[key] = (layout, nc)
    segs_per_core, NPOS, K, T, NB = layout

    wbd = _build_wrep(W)
    in_maps = []
    for c in range(NCORES):
        data = _pack_core(segs_per_core[c], NPOS, K, T, wbd,
                          edge_features, edge_sources, hidden)
        in_maps.append({"data": data})

    res = run_bass_kernel_spmd(nc, in_maps, list(range(NCORES)))
    outs = [res.results[c]["out"] for c in range(NCORES)]
    out = _assemble(outs, segs_per_core, NPOS, NB)

    if np.any(b):
        # bias term: out[n] += (sum_{e->n} hidden[src e]) @ Br,
        # Br[h, m] = b[m*16+h].  (b is all-zero for this problem.)
        Br = b.reshape(MSG, HID).T.astype(np.float32)
        acc = np.zeros((N_NODES, HID), dtype=np.float32)
        np.add.at(acc, edge_targets, hidden[edge_sources])
        out += acc @ Br
    return out
